# revision 34
# baseline (speedup 1.0000x reference)
"""Multi-head causal self-attention with RoPE on 8 Trainium2 NeuronCores.

Reference computation (B=2, S=2048, D=2048, H=16, DH=128):
    xs = hidden_q / sqrt(D)
    q,k,v = xs @ {Wq,Wk,Wv}.T        (reshaped to [B,H,S,DH])
    q,k <- RoPE(q,k)
    scores = q @ k.T / sqrt(DH)  (causal masked)
    p = softmax(scores); attn = p @ v
    out = (attn / sqrt(H*DH)) @ Wo.T

Sharding: 8 cores = 2 (batch) x 4 (head-groups of 4 heads).  Each core
computes its head-group's projections, attention and a partial output
projection; the host sums the 4 partials per batch.

All matmul inputs are bf16 (1 cyc/row on the PE, same as f32r, but
halves DMA/SBUF and enables DVE 2-byte fast modes + DMA-engine
transposes).  Softmax denominators are accumulated on the DVE and
summed across partitions by a gpsimd partition_all_reduce - no PE
denominator matmuls.  Q/K tiles are transposed by the DMA engines
(XBAR), not the PE.  Diagonal (causal-edge) blocks are computed on
sliced query ranges.  Projection / output-projection matmuls are
interleaved into the attention pipeline as PE filler so the tensor
engine queue never drains (keeps the PE p-state at max clock).
"""

import math
from collections import deque
from contextlib import ExitStack

import numpy as np
import ml_dtypes

import concourse.bass as bass
import concourse.bass_isa as bass_isa
import concourse.mybir as mybir
import concourse.tile as tile
from concourse.masks import make_identity
from concourse import bacc
from concourse.bass import ts
from concourse.bass_utils import run_bass_kernel_spmd

B, S, D, H, DH = 2, 2048, 2048, 16, 128
BASE = 10000.0
G = 4              # head-groups (cores per batch)
HG = H // G        # heads per group = 4
F = HG * DH        # features per group = 512
NT = S // 128      # 16 token tiles
NQB = S // 512     # 4 query blocks
F32 = mybir.dt.float32
BF16 = mybir.dt.bfloat16
SCALE = 1.0 / math.sqrt(DH)

_cache = {}


def _rope_tables():
    # [DH, S] tables: cosT[i, s] = cos(s * invfreq[i % 64])
    inv_freq = 1.0 / (BASE ** (np.arange(0, DH, 2, dtype=np.float64) / DH))
    t = np.arange(S, dtype=np.float64)
    freqs = np.outer(np.concatenate([inv_freq, inv_freq]), t)   # [DH, S]
    return (np.cos(freqs).astype(ml_dtypes.bfloat16),
            np.sin(freqs).astype(ml_dtypes.bfloat16))


def _swap_mat():
    # lhsT for swp = P.T @ q : swp[m, s] = sgn(m) * q[(m+64) % 128, s]
    p = np.zeros((128, 128), np.float32)
    for m in range(128):
        p[(m + 64) % 128, m] = -1.0 if m < 64 else 1.0
    return p.astype(ml_dtypes.bfloat16)


def _build(reps=1):
    key = ("nc", reps)
    if key in _cache:
        return _cache[key]
    nc = bacc.Bacc("TRN2", target_bir_lowering=False, debug=False, num_devices=8)

    # all inputs arrive host-pretiled partition-major and contiguous so every
    # load optimizes to ~128 big DMA descriptors (descgen runs on the
    # dispatching engine's sequencer - descriptor count is sequencer time)
    x_d = [nc.dram_tensor(f"x{c}", [128, NT, 512], BF16, kind="ExternalInput")
           for c in range(4)]
    wq_d = nc.dram_tensor("wq", [128, NT, F], BF16, kind="ExternalInput")
    wk_d = nc.dram_tensor("wk", [128, NT, F], BF16, kind="ExternalInput")
    wv_d = nc.dram_tensor("wv", [128, NT, F], BF16, kind="ExternalInput")
    wo_d = nc.dram_tensor("wo", [128, G, D], BF16, kind="ExternalInput")
    cos_d = nc.dram_tensor("cos", [128, S], BF16, kind="ExternalInput")
    sin_d = nc.dram_tensor("sin", [128, S], BF16, kind="ExternalInput")
    swp_d = nc.dram_tensor("swp", [128, 128], BF16, kind="ExternalInput")
    tri_d = nc.dram_tensor("tri", [128, 128], BF16, kind="ExternalInput")
    y = nc.dram_tensor("y", [S, D], BF16, kind="ExternalOutput")

    with tile.TileContext(nc) as tc, ExitStack() as ctx:
        pers = ctx.enter_context(tc.tile_pool(name="pers", bufs=1))
        xpool = ctx.enter_context(tc.tile_pool(name="xpool", bufs=2))
        stage = ctx.enter_context(tc.tile_pool(name="stage", bufs=3))
        stage2 = ctx.enter_context(tc.tile_pool(name="stage2", bufs=1))
        tmp2 = ctx.enter_context(tc.tile_pool(name="tmp2", bufs=2))
        ptp = ctx.enter_context(tc.tile_pool(name="ptp", bufs=8))
        yst = ctx.enter_context(tc.tile_pool(name="yst", bufs=3))
        ps_pp = ctx.enter_context(tc.tile_pool(name="ps_pp", bufs=3, space="PSUM"))
        ps_sc = ctx.enter_context(tc.tile_pool(name="ps_sc", bufs=2, space="PSUM"))
        ps_at = ctx.enter_context(tc.tile_pool(name="ps_at", bufs=2, space="PSUM"))
        ps_tr = ctx.enter_context(tc.tile_pool(name="ps_tr", bufs=1, space="PSUM"))

        # ---------------- persistent SBUF ----------------
        wq_sb = pers.tile([128, NT, F], BF16, tag="wq")
        wk_sb = pers.tile([128, NT, F], BF16, tag="wk")
        wv_sb = pers.tile([128, NT, F], BF16, tag="wv")
        wo_sb = pers.tile([128, G, D], BF16, tag="wo")
        cos_sb = pers.tile([128, S], BF16, tag="cos")
        sin_sb = pers.tile([128, S], BF16, tag="sin")
        swp_sb = pers.tile([128, 128], BF16, tag="swp")
        tri_sb = pers.tile([128, 128], BF16, tag="tri")
        identb = pers.tile([128, 128], BF16, tag="identb")
        qT_sb = pers.tile([128, HG, S], BF16, tag="qT")
        kT_sb = pers.tile([128, HG, S], BF16, tag="kT")
        v_sb = pers.tile([128, NT, F], BF16, tag="v")
        attn_sb = pers.tile([128, HG, S], BF16, tag="attn")

        x_tiles = {}

        def fetch_x(sblk, defer=False):
            xt = xpool.tile([128, NT, 512], BF16, tag="x", name="x")
            if not defer:
                nc.scalar.dma_start(xt[:], x_d[sblk].ap())
            x_tiles[sblk] = xt
            return xt

        # consumption order on sync: x0 and wq in 4-kt chunks (the first
        # projection units start after ~0.9MB), cos/sin (rope chases), wk,
        # wv.  x1..x3 and wo go via the scalar HWDGE queue so their bulk
        # transfers never sit ahead of the rope transposes on sync.
        x0 = fetch_x(0, defer=True)
        for c in range(4):
            nc.sync.dma_start(x0[:, ts(c, 4), :], x_d[0].ap()[:, ts(c, 4), :])
            nc.sync.dma_start(wq_sb[:, ts(c, 4), :], wq_d.ap()[:, ts(c, 4), :])
        nc.sync.dma_start(cos_sb[:], cos_d.ap())
        nc.sync.dma_start(sin_sb[:], sin_d.ap())
        nc.sync.dma_start(tri_sb[:], tri_d.ap())
        nc.sync.dma_start(swp_sb[:], swp_d.ap())
        idf = stage2.tile([128, 512], F32, tag="ta", name="idf")
        make_identity(nc, idf[:, 0:128])
        nc.vector.tensor_copy(identb[:], idf[:, 0:128])
        for w_sb, w_d in ((wk_sb, wk_d), (wv_sb, wv_d)):
            for c in range(4):
                nc.sync.dma_start(w_sb[:, ts(c, 4), :], w_d.ap()[:, ts(c, 4), :])
        xt1 = fetch_x(1, defer=True)
        nc.sync.dma_start(xt1[:], x_d[1].ap())
        nc.sync.dma_start(wo_sb[:], wo_d.ap())

        # ---------------- filler machinery ----------------
        filler = deque()
        pending_tr = []

        def flush_tr(keep=1):
            """Deferred rope for a transposed-projected q/k tile: one signed
            swap matmul (PE) plus three DVE ops writing the rotated tile
            straight into qT_sb/kT_sb.  Deferred >=1 group so the PE's swap
            matmul never waits on the ACT staging copy."""
            while len(pending_tr) > keep:
                name, h, sblk, sb = pending_tr.pop(0)
                dst = qT_sb if name == "q" else kT_sb
                swp = ps_tr.tile([128, 512], F32, tag="tr", name="tr")
                nc.tensor.matmul(swp[:], swp_sb[:], sb[:],
                                 start=True, stop=True)
                cs = cos_sb[:, ts(sblk, 512)]
                sn = sin_sb[:, ts(sblk, 512)]
                ta = stage2.tile([128, 512], BF16, tag="ta", name="ta")
                nc.vector.tensor_mul(ta[:], sb[:], cs)
                tb_ = stage2.tile([128, 512], BF16, tag="tb", name="tb")
                nc.vector.tensor_mul(tb_[:], swp[:], sn)
                nc.vector.tensor_add(dst[:, h, ts(sblk, 512)], ta[:], tb_[:])

        def pull(n):
            for _ in range(n):
                if filler:
                    filler.popleft()()

        def drain():
            while filler:
                filler.popleft()()

        # ---------------- projections of one 512-token block ----------------
        def add_proj_sblk(sblk):
            """Staggered projection groups for one 512-token block.  Q and K
            are projected TRANSPOSED (out [dh, s], lhsT = weight d-tile, rhs
            = x d-tile) so no PE transposes are needed; RoPE then runs in
            the dh-major layout using one signed-swap matmul per tile.  V is
            projected in natural [tokens, f] layout per 128-token tile.
            Each group is 8 filler units of 2 matmuls."""
            xq = x_tiles[sblk]

            def finish_qk(name, h, ps):
                # psum [dh, 512] f32 -> bf16 staging, then deferred rope
                sb = stage.tile([128, 512], BF16, tag="qsb", name="qsb")
                nc.scalar.copy(sb[:], ps[:])
                pending_tr.append((name, h, sblk, sb))

            def add_group(name, key, ps_args):
                st = {}

                def mk_unit(kt0, first, last):
                    def unit():
                        if first:
                            flush_tr()
                            st["ps"] = ps_pp.tile([128, 512], F32, tag="pp",
                                                  name="pp")
                        ps = st["ps"]
                        for kk in (kt0, kt0 + 1):
                            if name == "v":
                                tb = key
                                nc.tensor.matmul(
                                    ps[:], xq[:, kk, ts(tb % 4, 128)],
                                    wv_sb[:, kk, :],
                                    start=(kk == 0), stop=(kk == NT - 1))
                            else:
                                h = key
                                w_sb = wq_sb if name == "q" else wk_sb
                                nc.tensor.matmul(
                                    ps[:], w_sb[:, kk, ts(h, 128)],
                                    xq[:, kk, :],
                                    start=(kk == 0), stop=(kk == NT - 1))
                        if last:
                            if name == "v":
                                nc.vector.tensor_copy(v_sb[:, key, :], ps[:])
                            else:
                                finish_qk(name, key, ps)
                    return unit

                for u in range(8):
                    filler.append(mk_unit(2 * u, u == 0, u == 7))

            for name, key in (("q", 0), ("k", 0), ("q", 1), ("v", 0),
                              ("k", 1), ("q", 2), ("v", 1), ("k", 2),
                              ("q", 3), ("v", 2), ("k", 3), ("v", 3)):
                add_group(name, key if name != "v" else 4 * sblk + key,
                          None)

        # ---------------- output projection units ----------------
        def add_outproj_units(qb):
            """32 filler units (2 matmuls each): y partial for query block
            qb; each (qt, ddb) chunk is two units + copy/DMA chase."""
            st = {}

            def mk_unit(qt, ddb, first, last):
                def unit():
                    if first:
                        st["py"] = ps_pp.tile([128, 512], F32, tag="pp", name="pp")
                    py = st["py"]
                    fts = (0, 1) if first else (2, 3)
                    for ft in fts:
                        nc.tensor.matmul(py[:], attn_sb[:, ft, ts(qt, 128)],
                                         wo_sb[:, ft, ts(ddb, 512)],
                                         start=(ft == 0), stop=(ft == G - 1))
                    if last:
                        y_sb = yst.tile([128, 512], BF16, tag="ysb")
                        if qb == 3 and (qt + ddb) % 2 == 0:
                            nc.scalar.copy(y_sb[:], py[:])
                        else:
                            nc.vector.tensor_copy(y_sb[:], py[:])
                        yeng = nc.scalar if (qb == 3 and ddb % 2) else nc.sync
                        yeng.dma_start(y.ap()[ts(qt, 128), ts(ddb, 512)],
                                       y_sb[:])
                return unit

            for qt in range(4 * qb, 4 * qb + 4):
                for ddb in range(NQB):
                    filler.append(mk_unit(qt, ddb, True, False))
                    filler.append(mk_unit(qt, ddb, False, True))

        # ---------------- attention for one (qb, h) ----------------
        def attn_h(qb, h):
            nkt = 4 * qb + 4
            p_att = ps_at.tile([128, 512], F32, tag="att", name="att")
            den = tmp2.tile([128, 512], F32, tag="den", name="den")
            pts = {}
            t2 = {}

            def a_of(kt):
                q0 = max(0, 128 * (kt - 4 * qb))
                nc.tensor.matmul(p_att[:, q0:512], v_sb[:, kt, ts(h, 128)],
                                 pts[kt][:, q0:512],
                                 start=(kt == 0), stop=(kt == nkt - 1))

            for kt in range(nkt):
                j = kt - 4 * qb
                q0 = max(0, 128 * j)
                psc = ps_sc.tile([128, 512], F32, tag="sc", name="sc")
                nc.tensor.matmul(psc[:, q0:512],
                                 kT_sb[:, h, ts(kt, 128)],
                                 qT_sb[:, h, 512 * qb + q0:512 * (qb + 1)],
                                 start=True, stop=(j < 0))
                if j >= 0:
                    # additive causal bias on the 128-col diagonal block
                    nc.tensor.matmul(psc[:, ts(j, 128)], identb[:], tri_sb[:],
                                     start=False, stop=True)
                pt = ptp.tile([128, 512], BF16, tag="pt", name="pt")
                pts[kt] = pt
                nc.scalar.activation(pt[:, q0:512], psc[:, q0:512],
                                     mybir.ActivationFunctionType.Exp,
                                     scale=SCALE)
                if j >= 0:
                    if qb == 0 and kt == 0:
                        nc.vector.tensor_copy(den[:], pt[:])
                    else:
                        nc.vector.tensor_add(den[:, q0:], den[:, q0:],
                                             pt[:, q0:])
                else:
                    # off-diagonal: bf16 pair/quad tree, one f32 add per 4 kt
                    if kt % 2 == 1:
                        tt = tmp2.tile([128, 512], BF16, tag="t2", name="t2")
                        nc.vector.tensor_add(tt[:], pts[kt - 1][:], pt[:])
                        t2[kt // 2] = tt
                    if kt % 4 == 3:
                        t4 = tmp2.tile([128, 512], BF16, tag="t4", name="t4")
                        nc.vector.tensor_add(t4[:], t2[kt // 2 - 1][:],
                                             t2[kt // 2][:])
                        if kt == 3:
                            nc.vector.tensor_copy(den[:], t4[:])
                        else:
                            nc.vector.tensor_add(den[:], den[:], t4[:])
                pull(2 if kt % 2 == 0 else 1)
                if kt >= 2:
                    a_of(kt - 2)
            a_of(nkt - 2)
            a_of(nkt - 1)
            # normalize: cross-partition sum, reciprocal, scale
            rb = tmp2.tile([128, 512], F32, tag="rb")
            nc.gpsimd.partition_all_reduce(rb[:], den[:], 128,
                                           bass_isa.ReduceOp.add)
            rcp = tmp2.tile([128, 512], F32, tag="rcp")
            nc.vector.reciprocal_approx_fast(rcp[:], rb[:])
            nc.vector.tensor_mul(attn_sb[:, h, ts(qb, 512)], p_att[:], rcp[:])

        # ---------------- schedule ----------------
        add_proj_sblk(0)
        drain()
        for qb in range(NQB):
            if qb < 3:
                if qb < 2:
                    fetch_x(qb + 2)
                add_proj_sblk(qb + 1)
            else:
                add_outproj_units(0)
                add_outproj_units(1)
                add_outproj_units(2)
            flush_tr(keep=0)
            for h in range(HG):
                attn_h(qb, h)
            drain()
            flush_tr(keep=0)
        add_outproj_units(3)
        drain()

    nc.compile()
    _cache[key] = nc
    return nc


def _in_maps(hidden_q, Wq, Wk, Wv, Wo):
    bf = ml_dtypes.bfloat16

    def tile_p(a, groups, width):
        # [groups*128, width] -> [128, groups, width] contiguous
        return np.ascontiguousarray(
            a.reshape(groups, 128, width).transpose(1, 0, 2)).astype(bf)

    xs = (np.asarray(hidden_q, np.float32) / math.sqrt(D))
    # x per (batch, sblk): [128, NT, 512] with partition = d % 128
    xh = []
    for b in range(B):
        a = np.ascontiguousarray(xs[b].T)          # [D, S]
        a = a.reshape(NT, 128, 4, 512).transpose(1, 0, 2, 3)  # [128,kt,sblk,s]
        xh.append([np.ascontiguousarray(a[:, :, c, :]).astype(bf)
                   for c in range(4)])
    cos_h, sin_h = _rope_tables()                       # [DH, S] bf16
    swp = _swap_mat()
    # additive causal bias: 0 where q>=k (valid), -1e4 where q<k; added to
    # the raw scores before exp so masked entries underflow to exactly 0
    tri = np.where(np.tril(np.ones((128, 128), np.float32)).T > 0,
                   0.0, -1e4).astype(bf)
    wo_s = np.asarray(Wo, np.float32) / math.sqrt(H * DH)
    in_maps = []
    for c in range(8):
        b, g = c // G, c % G
        rows = slice(F * g, F * (g + 1))
        m = {f"x{i}": xh[b][i] for i in range(4)}
        m.update({
            "wq": tile_p(np.ascontiguousarray(np.asarray(Wq, np.float32)[rows, :].T), NT, F),
            "wk": tile_p(np.ascontiguousarray(np.asarray(Wk, np.float32)[rows, :].T), NT, F),
            "wv": tile_p(np.ascontiguousarray(np.asarray(Wv, np.float32)[rows, :].T), NT, F),
            "wo": tile_p(np.ascontiguousarray(wo_s[:, rows].T), G, D),
            "cos": cos_h, "sin": sin_h, "tri": tri, "swp": swp,
        })
        in_maps.append(m)
    return in_maps


def kernel(hidden_q, attention_mask, position_bias, Wq, Wk, Wv, Wo):
    hidden_q = np.asarray(hidden_q)
    assert hidden_q.shape == (B, S, D)
    in_maps = _in_maps(hidden_q, Wq, Wk, Wv, Wo)
    nc = _build()
    res = run_bass_kernel_spmd(nc, in_maps, core_ids=list(range(8)))
    _cache["last_results"] = res
    out = np.zeros((B, S, D), np.float32)
    for c in range(8):
        out[c // G] += res.results[c]["y"].astype(np.float32)
    return out


# revision 35
# speedup vs baseline: 1.0046x; 1.0046x over previous
"""Multi-head causal self-attention with RoPE on 8 Trainium2 NeuronCores.

Reference computation (B=2, S=2048, D=2048, H=16, DH=128):
    xs = hidden_q / sqrt(D)
    q,k,v = xs @ {Wq,Wk,Wv}.T        (reshaped to [B,H,S,DH])
    q,k <- RoPE(q,k)
    scores = q @ k.T / sqrt(DH)  (causal masked)
    p = softmax(scores); attn = p @ v
    out = (attn / sqrt(H*DH)) @ Wo.T

Sharding: 8 cores = 2 (batch) x 4 (head-groups of 4 heads).  Each core
computes its head-group's projections, attention and a partial output
projection; the host sums the 4 partials per batch.

All matmul inputs are bf16 (1 cyc/row on the PE, same as f32r, but
halves DMA/SBUF and enables DVE 2-byte fast modes + DMA-engine
transposes).  Softmax denominators are accumulated on the DVE and
summed across partitions by a gpsimd partition_all_reduce - no PE
denominator matmuls.  Q/K tiles are transposed by the DMA engines
(XBAR), not the PE.  Diagonal (causal-edge) blocks are computed on
sliced query ranges.  Projection / output-projection matmuls are
interleaved into the attention pipeline as PE filler so the tensor
engine queue never drains (keeps the PE p-state at max clock).
"""

import math
from collections import deque
from contextlib import ExitStack

import numpy as np
import ml_dtypes

import concourse.bass as bass
import concourse.bass_isa as bass_isa
import concourse.mybir as mybir
import concourse.tile as tile
from concourse.masks import make_identity
from concourse import bacc
from concourse.bass import ts
from concourse.bass_utils import run_bass_kernel_spmd

B, S, D, H, DH = 2, 2048, 2048, 16, 128
BASE = 10000.0
G = 4              # head-groups (cores per batch)
HG = H // G        # heads per group = 4
F = HG * DH        # features per group = 512
NT = S // 128      # 16 token tiles
NQB = S // 512     # 4 query blocks
F32 = mybir.dt.float32
BF16 = mybir.dt.bfloat16
SCALE = 1.0 / math.sqrt(DH)

_cache = {}


def _rope_tables():
    # [DH, S] tables: cosT[i, s] = cos(s * invfreq[i % 64])
    inv_freq = 1.0 / (BASE ** (np.arange(0, DH, 2, dtype=np.float64) / DH))
    t = np.arange(S, dtype=np.float64)
    freqs = np.outer(np.concatenate([inv_freq, inv_freq]), t)   # [DH, S]
    return (np.cos(freqs).astype(ml_dtypes.bfloat16),
            np.sin(freqs).astype(ml_dtypes.bfloat16))


def _swap_mat():
    # lhsT for swp = P.T @ q : swp[m, s] = sgn(m) * q[(m+64) % 128, s]
    p = np.zeros((128, 128), np.float32)
    for m in range(128):
        p[(m + 64) % 128, m] = -1.0 if m < 64 else 1.0
    return p.astype(ml_dtypes.bfloat16)


def _build(reps=1):
    key = ("nc", reps)
    if key in _cache:
        return _cache[key]
    nc = bacc.Bacc("TRN2", target_bir_lowering=False, debug=False, num_devices=8)

    # all inputs arrive host-pretiled partition-major and contiguous so every
    # load optimizes to ~128 big DMA descriptors (descgen runs on the
    # dispatching engine's sequencer - descriptor count is sequencer time)
    x_d = [nc.dram_tensor(f"x{c}", [128, NT, 512], BF16, kind="ExternalInput")
           for c in range(4)]
    wq_d = nc.dram_tensor("wq", [128, NT, F], BF16, kind="ExternalInput")
    wk_d = nc.dram_tensor("wk", [128, NT, F], BF16, kind="ExternalInput")
    wv_d = nc.dram_tensor("wv", [128, NT, F], BF16, kind="ExternalInput")
    wo_d = nc.dram_tensor("wo", [128, G, D], BF16, kind="ExternalInput")
    cos_d = nc.dram_tensor("cos", [128, S], BF16, kind="ExternalInput")
    sin_d = nc.dram_tensor("sin", [128, S], BF16, kind="ExternalInput")
    swp_d = nc.dram_tensor("swp", [128, 128], BF16, kind="ExternalInput")
    tri_d = nc.dram_tensor("tri", [128, 128], BF16, kind="ExternalInput")
    y = nc.dram_tensor("y", [S, D], BF16, kind="ExternalOutput")

    with tile.TileContext(nc) as tc, ExitStack() as ctx:
        pers = ctx.enter_context(tc.tile_pool(name="pers", bufs=1))
        xpool = ctx.enter_context(tc.tile_pool(name="xpool", bufs=2))
        stage = ctx.enter_context(tc.tile_pool(name="stage", bufs=3))
        stage2 = ctx.enter_context(tc.tile_pool(name="stage2", bufs=1))
        tmp2 = ctx.enter_context(tc.tile_pool(name="tmp2", bufs=2))
        ptp = ctx.enter_context(tc.tile_pool(name="ptp", bufs=8))
        yst = ctx.enter_context(tc.tile_pool(name="yst", bufs=3))
        ps_pp = ctx.enter_context(tc.tile_pool(name="ps_pp", bufs=3, space="PSUM"))
        ps_sc = ctx.enter_context(tc.tile_pool(name="ps_sc", bufs=2, space="PSUM"))
        ps_at = ctx.enter_context(tc.tile_pool(name="ps_at", bufs=2, space="PSUM"))
        ps_tr = ctx.enter_context(tc.tile_pool(name="ps_tr", bufs=1, space="PSUM"))

        # ---------------- persistent SBUF ----------------
        wq_sb = pers.tile([128, NT, F], BF16, tag="wq")
        wk_sb = pers.tile([128, NT, F], BF16, tag="wk")
        wv_sb = pers.tile([128, NT, F], BF16, tag="wv")
        wo_sb = pers.tile([128, G, D], BF16, tag="wo")
        cos_sb = pers.tile([128, S], BF16, tag="cos")
        sin_sb = pers.tile([128, S], BF16, tag="sin")
        swp_sb = pers.tile([128, 128], BF16, tag="swp")
        tri_sb = pers.tile([128, 128], BF16, tag="tri")
        identb = pers.tile([128, 128], BF16, tag="identb")
        qT_sb = pers.tile([128, HG, S], BF16, tag="qT")
        kT_sb = pers.tile([128, HG, S], BF16, tag="kT")
        v_sb = pers.tile([128, NT, F], BF16, tag="v")
        attn_sb = pers.tile([128, HG, S], BF16, tag="attn")

        x_tiles = {}

        def fetch_x(sblk, defer=False):
            xt = xpool.tile([128, NT, 512], BF16, tag="x", name="x")
            if not defer:
                nc.scalar.dma_start(xt[:], x_d[sblk].ap())
            x_tiles[sblk] = xt
            return xt

        # consumption order on sync: x0 and wq in 4-kt chunks (the first
        # projection units start after ~0.9MB), cos/sin (rope chases), wk,
        # wv.  x1..x3 and wo go via the scalar HWDGE queue so their bulk
        # transfers never sit ahead of the rope transposes on sync.
        x0 = fetch_x(0, defer=True)
        for c in range(4):
            nc.sync.dma_start(x0[:, ts(c, 4), :], x_d[0].ap()[:, ts(c, 4), :])
            nc.sync.dma_start(wq_sb[:, ts(c, 4), :], wq_d.ap()[:, ts(c, 4), :])
        nc.sync.dma_start(cos_sb[:], cos_d.ap())
        nc.sync.dma_start(sin_sb[:], sin_d.ap())
        nc.sync.dma_start(tri_sb[:], tri_d.ap())
        nc.sync.dma_start(swp_sb[:], swp_d.ap())
        idf = stage2.tile([128, 512], F32, tag="ta", name="idf")
        make_identity(nc, idf[:, 0:128])
        nc.vector.tensor_copy(identb[:], idf[:, 0:128])
        for w_sb, w_d in ((wk_sb, wk_d), (wv_sb, wv_d)):
            for c in range(4):
                nc.sync.dma_start(w_sb[:, ts(c, 4), :], w_d.ap()[:, ts(c, 4), :])
        xt1 = fetch_x(1, defer=True)
        nc.sync.dma_start(xt1[:], x_d[1].ap())
        nc.sync.dma_start(wo_sb[:], wo_d.ap())

        # ---------------- filler machinery ----------------
        filler = deque()
        pending_tr = []

        def flush_tr(keep=1):
            """Deferred rope for a transposed-projected q/k tile: one signed
            swap matmul (PE) plus three DVE ops writing the rotated tile
            straight into qT_sb/kT_sb.  Deferred >=1 group so the PE's swap
            matmul never waits on the ACT staging copy."""
            while len(pending_tr) > keep:
                name, h, sblk, sb = pending_tr.pop(0)
                dst = qT_sb if name == "q" else kT_sb
                swp = ps_tr.tile([128, 512], F32, tag="tr", name="tr")
                nc.tensor.matmul(swp[:], swp_sb[:], sb[:],
                                 start=True, stop=True)
                cs = cos_sb[:, ts(sblk, 512)]
                sn = sin_sb[:, ts(sblk, 512)]
                ta = stage2.tile([128, 512], BF16, tag="ta", name="ta")
                nc.vector.tensor_mul(ta[:], sb[:], cs)
                tb_ = stage2.tile([128, 512], BF16, tag="tb", name="tb")
                nc.vector.tensor_mul(tb_[:], swp[:], sn)
                nc.vector.tensor_add(dst[:, h, ts(sblk, 512)], ta[:], tb_[:])

        def pull(n):
            for _ in range(n):
                if filler:
                    filler.popleft()()

        def drain():
            while filler:
                filler.popleft()()

        # ---------------- projections of one 512-token block ----------------
        def add_proj_sblk(sblk):
            """Staggered projection groups for one 512-token block.  Q and K
            are projected TRANSPOSED (out [dh, s], lhsT = weight d-tile, rhs
            = x d-tile) so no PE transposes are needed; RoPE then runs in
            the dh-major layout using one signed-swap matmul per tile.  V is
            projected in natural [tokens, f] layout per 128-token tile.
            Each group is 8 filler units of 2 matmuls."""
            xq = x_tiles[sblk]

            def finish_qk(name, h, ps):
                # psum [dh, 512] f32 -> bf16 staging, then deferred rope
                sb = stage.tile([128, 512], BF16, tag="qsb", name="qsb")
                nc.scalar.copy(sb[:], ps[:])
                pending_tr.append((name, h, sblk, sb))

            def add_group(name, key, ps_args):
                st = {}

                def mk_unit(kt0, first, last):
                    def unit():
                        if first:
                            flush_tr()
                            st["ps"] = ps_pp.tile([128, 512], F32, tag="pp",
                                                  name="pp")
                        ps = st["ps"]
                        for kk in (kt0, kt0 + 1):
                            if name == "v":
                                tb = key
                                nc.tensor.matmul(
                                    ps[:], xq[:, kk, ts(tb % 4, 128)],
                                    wv_sb[:, kk, :],
                                    start=(kk == 0), stop=(kk == NT - 1))
                            else:
                                h = key
                                w_sb = wq_sb if name == "q" else wk_sb
                                nc.tensor.matmul(
                                    ps[:], w_sb[:, kk, ts(h, 128)],
                                    xq[:, kk, :],
                                    start=(kk == 0), stop=(kk == NT - 1))
                        if last:
                            if name == "v":
                                nc.vector.tensor_copy(v_sb[:, key, :], ps[:])
                            else:
                                finish_qk(name, key, ps)
                    return unit

                for u in range(8):
                    filler.append(mk_unit(2 * u, u == 0, u == 7))

            for name, key in (("q", 0), ("k", 0), ("q", 1), ("v", 0),
                              ("k", 1), ("q", 2), ("v", 1), ("k", 2),
                              ("q", 3), ("v", 2), ("k", 3), ("v", 3)):
                add_group(name, key if name != "v" else 4 * sblk + key,
                          None)

        # ---------------- output projection units ----------------
        def add_outproj_units(qb):
            """32 filler units (2 matmuls each): y partial for query block
            qb; each (qt, ddb) chunk is two units + copy/DMA chase."""
            st = {}

            def mk_unit(qt, ddb, first, last):
                def unit():
                    if first:
                        st["py"] = ps_pp.tile([128, 512], F32, tag="pp", name="pp")
                    py = st["py"]
                    fts = (0, 1) if first else (2, 3)
                    for ft in fts:
                        nc.tensor.matmul(py[:], attn_sb[:, ft, ts(qt, 128)],
                                         wo_sb[:, ft, ts(ddb, 512)],
                                         start=(ft == 0), stop=(ft == G - 1))
                    if last:
                        y_sb = yst.tile([128, 512], BF16, tag="ysb")
                        if qb == 3 and (qt + ddb) % 2 == 0:
                            nc.scalar.copy(y_sb[:], py[:])
                        else:
                            nc.vector.tensor_copy(y_sb[:], py[:])
                        yeng = nc.scalar if (qb == 3 and ddb % 2) else nc.sync
                        yeng.dma_start(y.ap()[ts(qt, 128), ts(ddb, 512)],
                                       y_sb[:])
                return unit

            for qt in range(4 * qb, 4 * qb + 4):
                for ddb in range(NQB):
                    filler.append(mk_unit(qt, ddb, True, False))
                    filler.append(mk_unit(qt, ddb, False, True))

        # ---------------- attention for one (qb, h) ----------------
        def attn_h(qb, h):
            nkt = 4 * qb + 4
            p_att = ps_at.tile([128, 512], F32, tag="att", name="att")
            den = tmp2.tile([128, 512], F32, tag="den", name="den")
            pts = {}
            t2 = {}

            def a_of(kt):
                q0 = max(0, 128 * (kt - 4 * qb))
                nc.tensor.matmul(p_att[:, q0:512], v_sb[:, kt, ts(h, 128)],
                                 pts[kt][:, q0:512],
                                 start=(kt == 0), stop=(kt == nkt - 1))

            for kt in range(nkt):
                j = kt - 4 * qb
                q0 = max(0, 128 * j)
                psc = ps_sc.tile([128, 512], F32, tag="sc", name="sc")
                nc.tensor.matmul(psc[:, q0:512],
                                 kT_sb[:, h, ts(kt, 128)],
                                 qT_sb[:, h, 512 * qb + q0:512 * (qb + 1)],
                                 start=True, stop=(j < 0))
                if j >= 0:
                    # additive causal bias on the 128-col diagonal block
                    nc.tensor.matmul(psc[:, ts(j, 128)], identb[:], tri_sb[:],
                                     start=False, stop=True)
                pt = ptp.tile([128, 512], BF16, tag="pt", name="pt")
                pts[kt] = pt
                nc.scalar.activation(pt[:, q0:512], psc[:, q0:512],
                                     mybir.ActivationFunctionType.Exp,
                                     scale=SCALE)
                if j >= 0:
                    if qb == 0 and kt == 0:
                        nc.vector.tensor_copy(den[:], pt[:])
                    else:
                        nc.vector.tensor_add(den[:, q0:], den[:, q0:],
                                             pt[:, q0:])
                else:
                    # off-diagonal: bf16 pair/quad tree, one f32 add per 4 kt
                    if kt % 2 == 1:
                        tt = tmp2.tile([128, 512], BF16, tag="t2", name="t2")
                        nc.vector.tensor_add(tt[:], pts[kt - 1][:], pt[:])
                        t2[kt // 2] = tt
                    if kt % 4 == 3:
                        t4 = tmp2.tile([128, 512], BF16, tag="t4", name="t4")
                        nc.vector.tensor_add(t4[:], t2[kt // 2 - 1][:],
                                             t2[kt // 2][:])
                        if kt == 3:
                            nc.vector.tensor_copy(den[:], t4[:])
                        else:
                            nc.vector.tensor_add(den[:], den[:], t4[:])
                pull(2 if kt % 2 == 0 else 1)
                if kt >= 2:
                    a_of(kt - 2)
            a_of(nkt - 2)
            a_of(nkt - 1)
            # normalize: cross-partition sum, reciprocal, scale
            rb = tmp2.tile([128, 512], F32, tag="rb")
            nc.gpsimd.partition_all_reduce(rb[:], den[:], 128,
                                           bass_isa.ReduceOp.add)
            rcp = tmp2.tile([128, 512], F32, tag="rcp")
            nc.vector.reciprocal_approx_fast(rcp[:], rb[:])
            nc.vector.tensor_mul(attn_sb[:, h, ts(qb, 512)], p_att[:], rcp[:])

        # ---------------- schedule ----------------
        add_proj_sblk(0)
        drain()
        for qb in range(NQB):
            if qb < 3:
                if qb < 2:
                    fetch_x(qb + 2)
                add_proj_sblk(qb + 1)
            else:
                add_outproj_units(0)
                add_outproj_units(1)
                add_outproj_units(2)
            flush_tr(keep=0)
            for h in range(HG):
                attn_h(qb, h)
            drain()
        add_outproj_units(3)
        drain()

    nc.compile()
    _cache[key] = nc
    return nc


def _in_maps(hidden_q, Wq, Wk, Wv, Wo):
    bf = ml_dtypes.bfloat16

    def tile_p(a, groups, width):
        # [groups*128, width] -> [128, groups, width] contiguous
        return np.ascontiguousarray(
            a.reshape(groups, 128, width).transpose(1, 0, 2)).astype(bf)

    xs = (np.asarray(hidden_q, np.float32) / math.sqrt(D))
    # x per (batch, sblk): [128, NT, 512] with partition = d % 128
    xh = []
    for b in range(B):
        a = np.ascontiguousarray(xs[b].T)          # [D, S]
        a = a.reshape(NT, 128, 4, 512).transpose(1, 0, 2, 3)  # [128,kt,sblk,s]
        xh.append([np.ascontiguousarray(a[:, :, c, :]).astype(bf)
                   for c in range(4)])
    cos_h, sin_h = _rope_tables()                       # [DH, S] bf16
    swp = _swap_mat()
    # additive causal bias: 0 where q>=k (valid), -1e4 where q<k; added to
    # the raw scores before exp so masked entries underflow to exactly 0
    tri = np.where(np.tril(np.ones((128, 128), np.float32)).T > 0,
                   0.0, -1e4).astype(bf)
    wo_s = np.asarray(Wo, np.float32) / math.sqrt(H * DH)
    in_maps = []
    for c in range(8):
        b, g = c // G, c % G
        rows = slice(F * g, F * (g + 1))
        m = {f"x{i}": xh[b][i] for i in range(4)}
        m.update({
            "wq": tile_p(np.ascontiguousarray(np.asarray(Wq, np.float32)[rows, :].T), NT, F),
            "wk": tile_p(np.ascontiguousarray(np.asarray(Wk, np.float32)[rows, :].T), NT, F),
            "wv": tile_p(np.ascontiguousarray(np.asarray(Wv, np.float32)[rows, :].T), NT, F),
            "wo": tile_p(np.ascontiguousarray(wo_s[:, rows].T), G, D),
            "cos": cos_h, "sin": sin_h, "tri": tri, "swp": swp,
        })
        in_maps.append(m)
    return in_maps


def kernel(hidden_q, attention_mask, position_bias, Wq, Wk, Wv, Wo):
    hidden_q = np.asarray(hidden_q)
    assert hidden_q.shape == (B, S, D)
    in_maps = _in_maps(hidden_q, Wq, Wk, Wv, Wo)
    nc = _build()
    res = run_bass_kernel_spmd(nc, in_maps, core_ids=list(range(8)))
    _cache["last_results"] = res
    out = np.zeros((B, S, D), np.float32)
    for c in range(8):
        out[c // G] += res.results[c]["y"].astype(np.float32)
    return out
